# revision 17
# baseline (speedup 1.0000x reference)
"""GAT layer (nn_GATConv) Trainium2 kernel — 8-core column-parallel.

Math (per head h):
    Wh = x @ W[h]            f1 = Wh @ a1[h]          f2 = Wh @ a2[h]
    e_ij = leaky_relu(f1_i + f2_j, 0.2)
    att = softmax_j(where(adj_ij > 0, e_ij, -9e15))
    out_h = att @ Wh

Key identity used here:  exp(leaky_relu(x)) = max(e^x, e^{0.2 x}), so with
u=exp(f1), p=exp(0.2 f1) (dest side, i) and v=exp(f2), q=exp(0.2 f2)
(source side, j):
    n_ij = adj_ij * max(u_i v_j, p_i q_j)        (unnormalized att; exact)
    out_h[i] = (sum_j n_ij Wh[j]) / (sum_j n_ij)
Softmax is shift-invariant and |f1+f2| is O(5), so skipping the row-max
subtraction is exact in fp32 (no overflow/underflow; masked entries are
exact zeros via the adj multiply, matching exp(-9e15 - m) == 0).

Sharding: core r owns source nodes j in [r*1024, (r+1)*1024) — it receives
adjT = adj[:, own].T (bf16, exact for a 0/1 matrix), xT = x[own].T and the
(host-packed) params. It computes its column-slice contribution
outT_partial[c, i] = sum_{j in own} Z[j, c] * n_ij with Z = [Wh | 1],
for all heads. A ReduceScatter over the 8 cores sums the partials and hands
core r the finished block of head r ([66, 8192]); each core then divides by
the denominator row and returns head r's [64, 8192] (host transposes).
"""

import sys

for _p in ("/opt/trn_rl_repo",):
    if _p not in sys.path:
        sys.path.insert(0, _p)

import numpy as np

import concourse.bass as bass
import concourse.bacc as bacc
import concourse.tile as tile
import concourse.mybir as mybir

N = 8192
FIN = 512
D = 64
H = 8
NC = 8
ALPHA = 0.2
JLOC = N // NC          # own source nodes per core
JT = JLOC // 128        # j partition-tiles per core
ICW = 1024              # i-chunk width of the main loop
NIC = N // ICW
ZC = D + 2
NT_DVE = 2           # adj-mult tiles handled by DVE; rest go to GPSIMD              # Z columns per head: [Wh(64) | ones | zero-pad]

F32 = mybir.dt.float32
F32R = mybir.dt.float32r
BF16 = mybir.dt.bfloat16
AF = mybir.ActivationFunctionType
OP = mybir.AluOpType

DT_E = F32  # elementwise dtype of the N^2 stage

RG = [list(range(NC))]


def build_bass():
    nc = bacc.Bacc("TRN2", target_bir_lowering=False, debug=False, num_devices=NC)

    adjt = nc.dram_tensor("adjt", [JLOC, N], F32, kind="ExternalInput")
    xt = nc.dram_tensor("xt", [FIN, JLOC], F32, kind="ExternalInput")
    wext = nc.dram_tensor("wext", [FIN, H * D + 2 * H], F32, kind="ExternalInput")
    ident_in = nc.dram_tensor("ident", [128, 128], F32, kind="ExternalInput")
    out = nc.dram_tensor("out", [D, N], F32, kind="ExternalOutput")

    with tile.TileContext(nc) as tc:
        with (
            tc.tile_pool(name="const", bufs=1) as constp,
            tc.tile_pool(name="psb", bufs=1, space="PSUM") as psb,
            tc.tile_pool(name="pso", bufs=1, space="PSUM") as pso,
            tc.tile_pool(name="dram", bufs=1, space="DRAM") as dramp,
        ):
            ones_row = constp.tile([1, 128], F32)
            nc.vector.memset(ones_row, 1.0)
            ident = constp.tile([128, 128], F32)
            nc.sync.dma_start(ident, ident_in[:])

            ag_in = dramp.tile([16, JLOC], F32)
            ag_out = dramp.tile([NC * 16, JLOC], F32)
            rs_ins = []
            rs_outs = []
            for ic in range(NIC):
                rsi = dramp.tile([H * 66, ICW], F32, name=f"rsi{ic}")
                rso = dramp.tile([66, ICW], F32, name=f"rso{ic}")
                rs_ins.append(rsi)
                rs_outs.append(rso)

            with (
                tc.tile_pool(name="zp", bufs=1) as zp,
                tc.tile_pool(name="band", bufs=2) as bandp,
                tc.tile_pool(name="bc", bufs=3) as bcp,
                tc.tile_pool(name="work", bufs=3) as workp,
            ):
                # ---- Step 1: Wh/f1/f2 for own nodes -------------------------
                xts = zp.tile([128, 4, JLOC], F32)
                nc.sync.dma_start(xts, xt.rearrange("(s p) j -> p s j", p=128))
                wes = zp.tile([128, 4, H * D + 2 * H], F32)
                nc.sync.dma_start(wes, wext.rearrange("(s p) c -> p s c", p=128))

                Z = zp.tile([128, JT, H, ZC], F32R)
                onestage = zp.tile([128, JT, H, 2], F32)
                nc.vector.memset(onestage[:, :, :, 0:1], 1.0)
                nc.vector.memset(onestage[:, :, :, 1:2], 0.0)
                nc.vector.tensor_copy(Z[:, :, :, D : D + 2], onestage)
                vq = zp.tile([128, JT, 2, H], F32)
                stage = zp.tile([128, JT, 16], F32)

                for jt in range(JT):
                    ps_wh = psb.tile([128, 1024], F32, tag="bld", bufs=2, name=f"pswh{jt}")
                    ps_f = psb.tile([128, 1024], F32, tag="bld2", name=f"psf{jt}")
                    for s in range(4):
                        nc.tensor.matmul(
                            ps_wh[:, 0:512],
                            xts[:, s, jt * 128 : (jt + 1) * 128],
                            wes[:, s, 0:512],
                            start=(s == 0),
                            stop=(s == 3),
                        )
                        nc.tensor.matmul(
                            ps_f[:, 0:16],
                            xts[:, s, jt * 128 : (jt + 1) * 128],
                            wes[:, s, 512:528],
                            start=(s == 0),
                            stop=(s == 3),
                        )
                    nc.vector.tensor_copy(
                        Z[:, jt, :, 0:D],
                        ps_wh[:, 0:512].rearrange("p (h d) -> p h d", h=H),
                    )
                    # v=exp(f2), q=exp(0.2 f2) (per-partition scalars later)
                    nc.scalar.activation(vq[:, jt, 0, :], ps_f[:, 8:16], AF.Exp)
                    nc.scalar.activation(
                        vq[:, jt, 1, :], ps_f[:, 8:16], AF.Exp, scale=ALPHA
                    )
                    # u=exp(f1), p=exp(0.2 f1) -> staged for AllGather
                    nc.scalar.activation(stage[:, jt, 0:8], ps_f[:, 0:8], AF.Exp)
                    nc.scalar.activation(
                        stage[:, jt, 8:16], ps_f[:, 0:8], AF.Exp, scale=ALPHA
                    )

                # ---- Step 2: AllGather the dest-side exp vectors -----------
                for jt in range(JT):
                    ps_t = psb.tile([16, 128], F32, tag="bld", bufs=2, name=f"pst{jt}")
                    nc.tensor.transpose(ps_t, stage[:, jt, :], ident)
                    st_t = workp.tile([16, 128], F32, tag="stt", name=f"stt{jt}")
                    nc.vector.tensor_copy(st_t, ps_t)
                    nc.sync.dma_start(ag_in[:, jt * 128 : (jt + 1) * 128], st_t)
                nc.gpsimd.collective_compute(
                    "AllGather",
                    OP.bypass,
                    replica_groups=RG,
                    ins=[ag_in[:]],
                    outs=[ag_out[:]],
                )


                # ---- Step 3: main loop -------------------------------------
                for ic in range(NIC):
                    bts = []
                    for jt in range(JT):
                        bt = bandp.tile([128, ICW], F32, tag=f"b{jt}", name=f"bt{ic}_{jt}")
                        nc.sync.dma_start(
                            bt,
                            adjt[jt * 128 : (jt + 1) * 128, ic * ICW : (ic + 1) * ICW],
                        )
                        bts.append(bt)
                    for h in range(H):
                        # u_bc/p_bc [128, ICW]: dest-side exp rows broadcast
                        # across partitions via ones-outer-product on PE.
                        # i-chunk ic == AG block of rank ic (ICW == JLOC)
                        su = workp.tile([1, ICW], F32, tag="su", name=f"su{ic}_{h}")
                        nc.sync.dma_start(su, ag_out[ic * 16 + h : ic * 16 + h + 1, :])
                        sp = workp.tile([1, ICW], F32, tag="sp", name=f"sp{ic}_{h}")
                        nc.sync.dma_start(
                            sp, ag_out[ic * 16 + 8 + h : ic * 16 + 9 + h, :]
                        )
                        ps_u = psb.tile([128, ICW], F32, tag="bld", bufs=2, name=f"psu{ic}_{h}")
                        ps_p = psb.tile([128, ICW], F32, tag="bld2", name=f"psp{ic}_{h}")
                        for k in range(ICW // 512):
                            nc.tensor.matmul(
                                ps_u[:, k * 512 : (k + 1) * 512],
                                ones_row,
                                su[0:1, k * 512 : (k + 1) * 512],
                                start=True,
                                stop=True,
                            )
                            nc.tensor.matmul(
                                ps_p[:, k * 512 : (k + 1) * 512],
                                ones_row,
                                sp[0:1, k * 512 : (k + 1) * 512],
                                start=True,
                                stop=True,
                            )
                        pbc = bcp.tile([128, ICW], F32, tag="pbc", name=f"pbc{ic}_{h}")
                        nc.scalar.activation(pbc, ps_p, AF.Copy)
                        po = pso.tile([66, ICW], F32, tag="out", name=f"po{ic}_{h}")
                        for jt in range(JT):
                            # m1 = u_i * v_j on the scalar engine (ACT), read
                            # straight from the PSUM broadcast, per-partition
                            # scale = v.
                            m1 = workp.tile(
                                [128, ICW], DT_E, tag="m1", name=f"m1_{ic}_{h}_{jt}"
                            )
                            nc.scalar.activation(
                                m1, ps_u, AF.Copy, scale=vq[:, jt, 0, h : h + 1]
                            )
                            # E = max(p_i * q_j, m1) on DVE
                            et = workp.tile(
                                [128, ICW], DT_E, tag="et", name=f"et{ic}_{h}_{jt}"
                            )
                            nc.vector.scalar_tensor_tensor(
                                et, pbc, vq[:, jt, 1, h : h + 1], m1, OP.mult, OP.max
                            )
                            # n = E * adjT — split across DVE / GPSIMD
                            nt = workp.tile(
                                [128, ICW], F32R, tag="nt", name=f"nt{ic}_{h}_{jt}"
                            )
                            tt_eng = (
                                nc.vector
                                if (jt < 2 or (jt == 2 and h < 3))
                                else nc.gpsimd
                            )
                            tt_eng.tensor_tensor(nt, et, bts[jt], OP.mult)
                            for k in range(ICW // 512):
                                nc.tensor.matmul(
                                    po[:, k * 512 : (k + 1) * 512],
                                    Z[:, jt, h, :],
                                    nt[:, k * 512 : (k + 1) * 512],
                                    start=(jt == 0),
                                    stop=(jt == JT - 1),
                                )
                        evb = workp.tile([66, ICW], F32, tag="evb", name=f"evb{ic}_{h}")
                        nc.scalar.activation(evb, po, AF.Copy)
                        nc.sync.dma_start(rs_ins[ic][h * 66 : (h + 1) * 66, :], evb)
                    nc.gpsimd.collective_compute(
                        "ReduceScatter",
                        OP.add,
                        replica_groups=RG,
                        ins=[rs_ins[ic][:]],
                        outs=[rs_outs[ic][:]],
                    )

            # ---- Step 4: finale — divide by softmax denominator ------------
            with tc.tile_pool(name="fin", bufs=2) as finp:
                nums = finp.tile([D, N], F32, bufs=1)
                dens = finp.tile([1, N], F32, bufs=1)
                for ic in range(NIC):
                    sl = slice(ic * ICW, (ic + 1) * ICW)
                    nc.sync.dma_start(nums[:, sl], rs_outs[ic][0:D, :])
                    nc.sync.dma_start(dens[:, sl], rs_outs[ic][D : D + 1, :])
                denr = finp.tile([1, N], F32, bufs=1)
                nc.vector.reciprocal(denr, dens)
                for c in range(NIC):
                    psd = psb.tile([D, ICW], F32, tag="bld", bufs=2, name=f"psd{c}")
                    for k in range(ICW // 512):
                        nc.tensor.matmul(
                            psd[:, k * 512 : (k + 1) * 512],
                            ones_row[0:1, 0:D],
                            denr[0:1, c * ICW + k * 512 : c * ICW + (k + 1) * 512],
                            start=True,
                            stop=True,
                        )
                    fin = finp.tile([D, ICW], F32, tag="fin")
                    nc.vector.tensor_tensor(
                        fin, nums[:, c * ICW : (c + 1) * ICW], psd, OP.mult
                    )
                    nc.sync.dma_start(out[:, c * ICW : (c + 1) * ICW], fin)

    nc.finalize()
    return nc


_CACHE = {}


def _built():
    if "nc" not in _CACHE:
        _CACHE["nc"] = build_bass()
    return _CACHE["nc"]


def _in_maps(x, adj, W, a1, a2):
    x = np.asarray(x, dtype=np.float32)
    adj = np.asarray(adj, dtype=np.float32)
    W = np.asarray(W, dtype=np.float32)
    a1 = np.asarray(a1, dtype=np.float32)
    a2 = np.asarray(a2, dtype=np.float32)

    wcat = np.ascontiguousarray(W.transpose(1, 0, 2).reshape(FIN, H * D))
    wa1 = np.einsum("hfd,hd->fh", W, a1).astype(np.float32)
    wa2 = np.einsum("hfd,hd->fh", W, a2).astype(np.float32)
    wext = np.ascontiguousarray(np.concatenate([wcat, wa1, wa2], axis=1))
    ident = np.eye(128, dtype=np.float32)

    maps = []
    for r in range(NC):
        sl = slice(r * JLOC, (r + 1) * JLOC)
        maps.append(
            {
                "adjt": np.ascontiguousarray(adj[:, sl].T),
                "xt": np.ascontiguousarray(x[sl].T),
                "wext": wext,
                "ident": ident,
            }
        )
    return maps


def kernel(x, adj, W, a1, a2):
    from concourse.bass_utils import run_bass_kernel_spmd

    nc = _built()
    maps = _in_maps(x, adj, W, a1, a2)
    res = run_bass_kernel_spmd(nc, maps, core_ids=list(range(NC)))
    _CACHE["last_result"] = res
    full = np.empty((N, H * D), dtype=np.float32)
    for h in range(H):
        full[:, h * D : (h + 1) * D] = res.results[h]["out"].T
    return full
